# revision 4
# baseline (speedup 1.0000x reference)
"""Trainium2 Bass kernel for nn_Attention (B=4, N=2048, C=768, H=12, D=64).

Sharding: 8 cores = 4 batches x 2 head-groups (6 heads each).
Per core (all on-chip, bf16 matmuls, fp32 accumulation):
  qkT = (w_qk @ x^T)           [768, 2048]  (q rows pre-scaled by D^-0.5)
  v   = x @ w_v^T              [2048, 384]  (+ ones column per head)
  per head h, q-strip s(512):
    sT[kv,q] = kT_h^T-contract-d (K=64 matmuls, row-tiled head pairs)
    expS = exp(sT)  (no max subtraction: scores are O(1) by construction)
    av[q,65] = sum_kv expS^T-contract  (col 64 = softmax denominator)
    attn = av[:, :64] / av[:, 64]
    attnT via PE transpose
  y_part = attnT^T-contract @ w_p  [2048, 768]
Host: y[b] = y_part(group0) + y_part(group1) + b_proj.
"""

import sys

if "/opt/trn_rl_repo" not in sys.path:
    sys.path.insert(0, "/opt/trn_rl_repo")

import numpy as np
import ml_dtypes

import concourse.bacc as bacc
import concourse.mybir as mybir
import concourse.tile as tile
from concourse.masks import make_identity

FP32 = mybir.dt.float32
BF16 = mybir.dt.bfloat16
AF = mybir.ActivationFunctionType

DIM = 768
NUM_HEADS = 12
HEAD_DIM = 64
SCALE = HEAD_DIM ** -0.5
B, N = 4, 2048
HG = 6               # heads per core (head group)
CC = DIM // 128      # contraction chunks for qkv (6)
PAIRS = HG // 2      # head pairs per core (3)
S = N // 512         # q strips (4)
J = N // 128         # kv blocks (16)
CH = 3               # kv blocks per score psum chunk (3 banks)

_CACHED = {}


def build_core_program(reps=0):
    """One NeuronCore's program (SPMD: same program on all 8 cores).

    reps>0 wraps the compute body in a For_i hardware loop (timing builds)."""
    nc = bacc.Bacc("TRN2", debug=False, target_bir_lowering=False, num_devices=1)

    xt_d = nc.dram_tensor("xt", [DIM, N], BF16, kind="ExternalInput")
    wqk_d = nc.dram_tensor("wqk", [DIM, DIM], BF16, kind="ExternalInput")
    wv_d = nc.dram_tensor("wv", [DIM, HG * 64], BF16, kind="ExternalInput")
    wp_d = nc.dram_tensor("wp", [HG * 64, DIM], BF16, kind="ExternalInput")
    y_d = nc.dram_tensor("y", [N, DIM], FP32, kind="ExternalOutput")

    with tile.TileContext(nc) as tc:
        with (
            tc.tile_pool(name="persist", bufs=1) as persist,
            tc.tile_pool(name="exps", bufs=18) as exps_pool,
            tc.tile_pool(name="attnt", bufs=2) as attnt_pool,
            tc.tile_pool(name="small", bufs=4) as small_pool,
            tc.tile_pool(name="ysb", bufs=3) as y_pool,
            tc.tile_pool(name="ps_score", bufs=2, space="PSUM") as ps_score,
            tc.tile_pool(name="ps_small", bufs=2, space="PSUM") as ps_small,
        ):
            # ---- persistent SBUF ----
            xT = persist.tile([128, CC, N], BF16)          # [c, n] chunked
            wqk = persist.tile([128, CC, DIM], BF16)       # [c, o] (o: 384q+384k)
            wv = persist.tile([128, CC, HG * 64], BF16)    # [c, ov]
            wp = persist.tile([128, PAIRS, DIM], BF16)     # [c', o]
            qkT = persist.tile([128, CC, N], BF16)         # [o, n]
            v = persist.tile([128, J, HG, 65], BF16)       # [kv, j, h, d+1]
            ident = persist.tile([128, 128], BF16)

            nc.sync.dma_start(out=xT, in_=xt_d.ap().rearrange("(o p) n -> p o n", p=128))
            nc.sync.dma_start(out=wqk, in_=wqk_d.ap().rearrange("(o p) n -> p o n", p=128))
            nc.sync.dma_start(out=wv, in_=wv_d.ap().rearrange("(o p) n -> p o n", p=128))
            nc.sync.dma_start(out=wp, in_=wp_d.ap().rearrange("(o p) n -> p o n", p=128))
            make_identity(nc, ident)
            nc.vector.memset(v, 1.0)  # ones in col 64 of every head survive

            def qkv_pair(p):
                """Project qT,kT (o-tiles p and 3+p) and v for head pair p."""
                for ot in (p, PAIRS + p):
                    for s in range(S):
                        ps = ps_small.tile([128, 512], FP32, tag="sm")
                        for cc in range(CC):
                            nc.tensor.matmul(
                                ps,
                                wqk[:, cc, ot * 128 : ot * 128 + 128],
                                xT[:, cc, s * 512 : s * 512 + 512],
                                start=(cc == 0), stop=(cc == CC - 1),
                            )
                        nc.vector.tensor_copy(
                            out=qkT[:, ot, s * 512 : s * 512 + 512], in_=ps
                        )
                for nt in range(J):
                    psv = ps_small.tile([128, 512], FP32, tag="sm")
                    for cc in range(CC):
                        nc.tensor.matmul(
                            psv[:, 0:128],
                            xT[:, cc, nt * 128 : nt * 128 + 128],
                            wv[:, cc, p * 128 : p * 128 + 128],
                            start=(cc == 0), stop=(cc == CC - 1),
                        )
                    for h2 in range(2):
                        nc.vector.tensor_copy(
                            out=v[:, nt, 2 * p + h2, 0:64],
                            in_=psv[:, h2 * 64 : h2 * 64 + 64],
                        )

            def proj(s, attnT):
                """y[s*512 : (s+1)*512, :] = attnT^T @ wp."""
                for nt in range(4):
                    ysb = y_pool.tile([128, DIM], FP32, tag="y")
                    for og, ow in ((0, 512), (512, 256)):
                        psy = ps_small.tile([128, 512], FP32, tag="sm")
                        for cc in range(PAIRS):
                            nc.tensor.matmul(
                                psy[:, 0:ow],
                                attnT[:, cc, nt * 128 : nt * 128 + 128],
                                wp[:, cc, og : og + ow],
                                start=(cc == 0), stop=(cc == PAIRS - 1),
                            )
                        nc.vector.tensor_copy(out=ysb[:, og : og + ow], in_=psy[:, 0:ow])
                    row = s * 512 + nt * 128
                    nc.sync.dma_start(out=y_d.ap()[row : row + 128, :], in_=ysb)

            # chunking of the 16 kv blocks into score-psum chunks
            chunks = []
            j0 = 0
            while j0 < J:
                ln = min(CH, J - j0)
                chunks.append((j0, ln))
                j0 += ln

            def body():
                qkv_pair(0)
                qkv_pair(1)

                attnT_tiles = {}
                for s in range(S):
                    for hp in range(PAIRS):
                        if hp == 0:
                            attnT = attnt_pool.tile(
                                [128, PAIRS, 512], BF16, tag="attnT"
                            )
                            attnT_tiles[s] = attnT
                        attnT = attnT_tiles[s]

                        # --- QK^T + exp, chunked over kv ---
                        expS = {0: [], 1: []}
                        for (j0, ln) in chunks:
                            for h2, base in ((0, 0), (1, 64)):
                                pss = ps_score.tile(
                                    [128, 512 * CH], FP32, tag="sc"
                                )
                                for jj in range(ln):
                                    j = j0 + jj
                                    nc.tensor.matmul(
                                        pss[:, jj * 512 : jj * 512 + 512],
                                        qkT[base : base + 64, PAIRS + hp,
                                            j * 128 : j * 128 + 128],
                                        qkT[base : base + 64, hp,
                                            s * 512 : s * 512 + 512],
                                        start=True, stop=True,
                                        tile_position=(base, 0),
                                    )
                                et = exps_pool.tile(
                                    [128, 512 * CH], BF16, tag="e"
                                )
                                nc.scalar.activation(
                                    out=et[:, : 512 * ln],
                                    in_=pss[:, : 512 * ln],
                                    func=AF.Exp,
                                )
                                expS[h2].append((j0, ln, et))

                        # pipelined heavy PE work while ACT runs exp:
                        if s == 0 and hp < PAIRS - 1:
                            qkv_pair(hp + 1)
                        if hp == 0 and s >= 1:
                            proj(s - 1, attnT_tiles.pop(s - 1))

                        # --- AV + divide ---
                        attn_pair = small_pool.tile([128, 4, 128], BF16, tag="ap")
                        for h2 in range(2):
                            h = 2 * hp + h2
                            pav = ps_small.tile([128, 512], FP32, tag="sm")
                            for i in range(4):
                                for (j0, ln, et) in expS[h2]:
                                    for jj in range(ln):
                                        j = j0 + jj
                                        nc.tensor.matmul(
                                            pav[:, i * 128 : i * 128 + 65],
                                            et[:, jj * 512 + i * 128
                                               : jj * 512 + i * 128 + 128],
                                            v[:, j, h, :],
                                            start=(j == 0), stop=(j == J - 1),
                                        )
                            pav4 = pav.rearrange("p (r c) -> p r c", r=4)
                            rsb = small_pool.tile([128, 4], FP32, tag="r")
                            nc.vector.reciprocal(out=rsb, in_=pav4[:, :, 64])
                            nc.vector.tensor_tensor(
                                attn_pair[:, :, h2 * 64 : h2 * 64 + 64],
                                pav4[:, :, 0:64],
                                rsb[:, :, None].to_broadcast((128, 4, 64)),
                                mybir.AluOpType.mult,
                            )

                        # --- transpose pair block into attnT ---
                        for i in range(4):
                            pst = ps_small.tile([128, 512], BF16, tag="sm")
                            nc.tensor.transpose(
                                pst[:, 0:128], attn_pair[:, i, :], ident
                            )
                            nc.vector.tensor_copy(
                                out=attnT[:, hp, i * 128 : i * 128 + 128],
                                in_=pst[:, 0:128],
                            )

                proj(S - 1, attnT_tiles.pop(S - 1))

            from contextlib import nullcontext
            with (tc.For_i(0, reps, 1) if reps else nullcontext()):
                body()

    nc.compile()
    return nc


def _host_prep(x, w_qkv, w_proj):
    """Slice/transpose/cast inputs per core. Core c = 2*b + hg."""
    bf16 = ml_dtypes.bfloat16
    in_maps = []
    for c in range(8):
        b, hg = c // 2, c % 2
        r0 = 384 * hg
        wq = w_qkv[r0 : r0 + 384] * SCALE          # [384, 768] scaled q rows
        wk = w_qkv[768 + r0 : 768 + r0 + 384]
        wv = w_qkv[1536 + r0 : 1536 + r0 + 384]
        wqk = np.concatenate([wq, wk], axis=0)     # [768, 768]
        in_maps.append({
            "xt": np.ascontiguousarray(x[b].T).astype(bf16),
            "wqk": np.ascontiguousarray(wqk.T).astype(bf16),
            "wv": np.ascontiguousarray(wv.T).astype(bf16),
            "wp": np.ascontiguousarray(w_proj[:, r0 : r0 + 384].T).astype(bf16),
        })
    return in_maps


def _get_fn():
    """Build the Bass program and a cached jit callable over 8 cores."""
    if "fn" in _CACHED:
        return _CACHED["fn"]

    import jax
    from jax.sharding import Mesh, PartitionSpec
    from jax.experimental.shard_map import shard_map
    from concourse import bass2jax
    from concourse.bass2jax import _bass_exec_p, install_neuronx_cc_hook

    install_neuronx_cc_hook()
    nc = build_core_program()

    in_names = ["xt", "wqk", "wv", "wp"]
    out_avals = [jax.core.ShapedArray((N, DIM), np.float32)]
    partition_name = nc.partition_id_tensor.name if nc.partition_id_tensor else None

    def _body(xt, wqk, wv, wp, yzero):
        operands = [xt, wqk, wv, wp, yzero]
        names = in_names + ["y"]
        if nc.dbg_addr is not None:
            operands.append(np.zeros((1, 2), np.uint32))
            names.append(nc.dbg_addr.name)
        if partition_name is not None:
            operands.append(bass2jax.partition_id_tensor())
            names.append(partition_name)
        outs = _bass_exec_p.bind(
            *operands,
            out_avals=tuple(out_avals),
            in_names=tuple(names),
            out_names=("y",),
            lowering_input_output_aliases=(),
            sim_require_finite=True,
            sim_require_nnan=True,
            nc=nc,
        )
        return outs[0]

    devices = jax.devices()[:8]
    mesh = Mesh(np.asarray(devices), ("core",))
    fn = jax.jit(
        shard_map(
            _body, mesh=mesh,
            in_specs=(PartitionSpec("core"),) * 5,
            out_specs=PartitionSpec("core"),
            check_rep=False,
        ),
        keep_unused=True,
    )
    _CACHED["fn"] = fn
    return fn


def _run(in_maps):
    import jax

    fn = _get_fn()
    concat_in = [
        np.concatenate([m[name] for m in in_maps], axis=0)
        for name in ["xt", "wqk", "wv", "wp"]
    ]
    yzero = np.zeros((8 * N, DIM), np.float32)
    out = jax.block_until_ready(fn(*concat_in, yzero))
    return np.asarray(out).reshape(8, N, DIM)


def kernel(x, w_qkv, w_proj, b_proj):
    x = np.asarray(x, dtype=np.float32)
    w_qkv = np.asarray(w_qkv, dtype=np.float32)
    w_proj = np.asarray(w_proj, dtype=np.float32)
    b_proj = np.asarray(b_proj, dtype=np.float32)

    in_maps = _host_prep(x, w_qkv, w_proj)
    parts = _run(in_maps)

    y = np.empty((B, N, DIM), dtype=np.float32)
    for b in range(B):
        y[b] = parts[2 * b] + parts[2 * b + 1] + b_proj
    return y


# revision 7
# speedup vs baseline: 1.1018x; 1.1018x over previous
"""V2 Trainium2 Bass kernel for nn_Attention. See kernel.py for the math.

V2 structural changes vs V1:
- AV matmuls interleaved into the QK^T/exp chunk stream (PE no longer blocks
  on ACT; ACT has no tail idle).
- Score chunks CH=2 ([128,1024] psum, 2 banks) => banks: scores 2x2, AV 2,
  misc 2.
- qkv projection / output projection emitted as small "filler" groups spread
  across chunk steps.
"""

import sys

if "/opt/trn_rl_repo" not in sys.path:
    sys.path.insert(0, "/opt/trn_rl_repo")

import numpy as np
import ml_dtypes

import concourse.bacc as bacc
import concourse.mybir as mybir
import concourse.tile as tile
from concourse.masks import make_identity

FP32 = mybir.dt.float32
BF16 = mybir.dt.bfloat16
AF = mybir.ActivationFunctionType

DIM = 768
HEAD_DIM = 64
SCALE = HEAD_DIM ** -0.5
B, N = 4, 2048
HG = 6
CC = DIM // 128
PAIRS = HG // 2
S = N // 512
J = N // 128
CH = 2                      # kv blocks per score chunk
NCHUNK = J // CH            # 8 chunks per (head, strip)

_CACHED = {}


def build_core_program(reps=0, ablate=()):
    nc = bacc.Bacc("TRN2", debug=False, target_bir_lowering=False, num_devices=1)

    xt_d = nc.dram_tensor("xt", [DIM, N], BF16, kind="ExternalInput")
    wqk_d = nc.dram_tensor("wqk", [DIM, DIM], BF16, kind="ExternalInput")
    wv_d = nc.dram_tensor("wv", [DIM, HG * 64], BF16, kind="ExternalInput")
    wp_d = nc.dram_tensor("wp", [HG * 64, DIM], BF16, kind="ExternalInput")
    y_d = nc.dram_tensor("y", [N, DIM], FP32, kind="ExternalOutput")

    with tile.TileContext(nc) as tc:
        with (
            tc.tile_pool(name="persist", bufs=1) as persist,
            tc.tile_pool(name="exps", bufs=22) as exps_pool,
            tc.tile_pool(name="attnt", bufs=5) as attnt_pool,
            tc.tile_pool(name="small", bufs=4) as small_pool,
            tc.tile_pool(name="ysb", bufs=3) as y_pool,
            tc.tile_pool(name="ps_score", bufs=2, space="PSUM") as ps_score,
            tc.tile_pool(name="ps_av", bufs=2, space="PSUM") as ps_av,
            tc.tile_pool(name="ps_misc", bufs=2, space="PSUM") as ps_misc,
        ):
            xT = persist.tile([128, CC, N], BF16)
            wqk = persist.tile([128, CC, DIM], BF16)
            wv = persist.tile([128, CC, HG * 64], BF16)
            wp = persist.tile([128, PAIRS, DIM], BF16)
            qkT = persist.tile([128, CC, N], BF16)
            v = persist.tile([128, J, HG, 65], BF16)
            ident = persist.tile([128, 128], BF16)

            xt_r = xt_d.ap().rearrange("(o p) n -> p o n", p=128)
            wqk_r = wqk_d.ap().rearrange("(o p) n -> p o n", p=128)
            wv_r = wv_d.ap().rearrange("(o p) n -> p o n", p=128)
            for cc in range(CC):
                nc.sync.dma_start(out=wqk[:, cc], in_=wqk_r[:, cc])
                nc.sync.dma_start(out=xT[:, cc], in_=xt_r[:, cc])
                nc.sync.dma_start(out=wv[:, cc], in_=wv_r[:, cc])
            nc.sync.dma_start(out=wp, in_=wp_d.ap().rearrange("(o p) n -> p o n", p=128))
            make_identity(nc, ident)
            nc.vector.memset(v, 1.0)

            # ---------- filler groups (each: one psum-group of work) ----------

            def qk_group(ot, s):
                ps = ps_misc.tile([128, 512], FP32, tag="m")
                for cc in range(CC):
                    nc.tensor.matmul(
                        ps,
                        wqk[:, cc, ot * 128 : ot * 128 + 128],
                        xT[:, cc, s * 512 : s * 512 + 512],
                        start=(cc == 0), stop=(cc == CC - 1),
                    )
                nc.vector.tensor_copy(out=qkT[:, ot, s * 512 : s * 512 + 512], in_=ps)

            def v_group(p, nt):
                psv = ps_misc.tile([128, 512], FP32, tag="m")
                for cc in range(CC):
                    nc.tensor.matmul(
                        psv[:, 0:128],
                        xT[:, cc, nt * 128 : nt * 128 + 128],
                        wv[:, cc, p * 128 : p * 128 + 128],
                        start=(cc == 0), stop=(cc == CC - 1),
                    )
                for h2 in range(2):
                    nc.vector.tensor_copy(
                        out=v[:, nt, 2 * p + h2, 0:64],
                        in_=psv[:, h2 * 64 : h2 * 64 + 64],
                    )

            def proj_group(s, attnT, nt, og, ow, ysb):
                psy = ps_misc.tile([128, 512], FP32, tag="m")
                for cc in range(PAIRS):
                    nc.tensor.matmul(
                        psy[:, 0:ow],
                        attnT[:, cc, nt * 128 : nt * 128 + 128],
                        wp[:, cc, og : og + ow],
                        start=(cc == 0), stop=(cc == PAIRS - 1),
                    )
                nc.vector.tensor_copy(out=ysb[:, og : og + ow], in_=psy[:, 0:ow])
                if og == 512:
                    row = s * 512 + nt * 128
                    nc.sync.dma_start(out=y_d.ap()[row : row + 128, :], in_=ysb)

            def qkv_pair_fillers(p):
                out = []
                for ot in (p, PAIRS + p):
                    for s in range(S):
                        out.append(lambda ot=ot, s=s: qk_group(ot, s))
                for nt in range(J):
                    out.append(lambda nt=nt: v_group(p, nt))
                return out

            def proj_fillers(s, attnT):
                out = []
                for nt in range(4):
                    ysb = y_pool.tile([128, DIM], FP32, tag="y")
                    for og, ow in ((0, 512), (512, 256)):
                        out.append(
                            lambda nt=nt, og=og, ow=ow, ysb=ysb:
                                proj_group(s, attnT, nt, og, ow, ysb)
                        )
                return out

            def body():
                # pair-0 projections up front (lead-in; ACT idle here)
                for f in qkv_pair_fillers(0):
                    f()

                filler = []
                attnT_tiles = {}
                for hp in range(PAIRS):
                    if hp + 1 < PAIRS:
                        filler.extend(qkv_pair_fillers(hp + 1))
                    for s in range(S):
                        if hp == 0:
                            at = attnt_pool.tile(
                                [128, PAIRS, 512], BF16, tag="attnT",
                                name=f"attnT{s}",
                            )
                            attnT_tiles[s] = at
                        attnT = attnT_tiles[s]

                        pav = {}
                        for h2 in range(2):
                            pav[h2] = ps_av.tile(
                                [128, 512], FP32, tag="av", name=f"pav{h2}"
                            )

                        expS = {0: [None] * NCHUNK, 1: [None] * NCHUNK}

                        def emit_qkt_exp(c, s=s, hp=hp, expS=expS):
                            for h2, base in ((0, 0), (1, 64)):
                                pss = ps_score.tile(
                                    [128, 512 * CH], FP32, tag="sc"
                                )
                                for jj in range(CH):
                                    j = c * CH + jj
                                    if "qkt" in ablate:
                                        continue
                                    nc.tensor.matmul(
                                        pss[:, jj * 512 : jj * 512 + 512],
                                        qkT[base : base + 64, PAIRS + hp,
                                            j * 128 : j * 128 + 128],
                                        qkT[base : base + 64, hp,
                                            s * 512 : s * 512 + 512],
                                        start=True, stop=True,
                                        tile_position=(base, 0),
                                    )
                                et = exps_pool.tile(
                                    [128, 512 * CH], BF16, tag="e"
                                )
                                if "exp" not in ablate:
                                    nc.scalar.activation(
                                        out=et, in_=pss, func=AF.Exp,
                                    )
                                else:
                                    nc.vector.memset(et[:, 0:4], 1.0)
                                expS[h2][c] = et

                        def emit_av(c, hp=hp, pav=pav, expS=expS):
                            if "av" in ablate:
                                return
                            for h2 in range(2):
                                h = 2 * hp + h2
                                et = expS[h2][c]
                                for jj in range(CH):
                                    j = c * CH + jj
                                    nc.tensor.matmul(
                                        pav[h2][0:65, :],
                                        v[:, j, h, :],
                                        et[:, jj * 512 : jj * 512 + 512],
                                        start=(j == 0), stop=(j == J - 1),
                                    )

                        for c in range(NCHUNK):
                            emit_qkt_exp(c)
                            if c > 0:
                                emit_av(c - 1)
                            if filler:
                                filler.pop(0)()
                        emit_av(NCHUNK - 1)

                        # divide: attnT[c', q] = outT / D (D broadcast over
                        # the 64 head-dim partitions via gpsimd)
                        for h2 in range(2):
                            dsb = small_pool.tile([1, 512], FP32, tag="d")
                            if "av" in ablate:
                                nc.vector.memset(dsb, 1.0)
                            else:
                                nc.vector.reciprocal(
                                    out=dsb, in_=pav[h2][64:65, :]
                                )
                            rb = small_pool.tile([64, 512], FP32, tag="rb")
                            nc.gpsimd.partition_broadcast(rb, dsb, channels=64)
                            nc.vector.tensor_tensor(
                                attnT[h2 * 64 : h2 * 64 + 64, hp, :],
                                pav[h2][0:64, :],
                                rb,
                                mybir.AluOpType.mult,
                            )

                        if hp == PAIRS - 1:
                            filler.extend(
                                proj_fillers(s, attnT_tiles.pop(s))
                            )

                # drain remaining fillers (tail projections)
                for f in filler:
                    f()

            from contextlib import nullcontext
            with (tc.For_i(0, reps, 1) if reps else nullcontext()):
                body()

    nc.compile()
    return nc


def _host_prep(x, w_qkv, w_proj):
    bf16 = ml_dtypes.bfloat16
    in_maps = []
    for c in range(8):
        b, hg = c // 2, c % 2
        r0 = 384 * hg
        wq = w_qkv[r0 : r0 + 384] * SCALE
        wk = w_qkv[768 + r0 : 768 + r0 + 384]
        wvv = w_qkv[1536 + r0 : 1536 + r0 + 384]
        wqk = np.concatenate([wq, wk], axis=0)
        in_maps.append({
            "xt": np.ascontiguousarray(x[b].T).astype(bf16),
            "wqk": np.ascontiguousarray(wqk.T).astype(bf16),
            "wv": np.ascontiguousarray(wvv.T).astype(bf16),
            "wp": np.ascontiguousarray(w_proj[:, r0 : r0 + 384].T).astype(bf16),
        })
    return in_maps


def _get_fn():
    if "fn" in _CACHED:
        return _CACHED["fn"]

    import jax
    from jax.sharding import Mesh, PartitionSpec
    from jax.experimental.shard_map import shard_map
    from concourse import bass2jax
    from concourse.bass2jax import _bass_exec_p, install_neuronx_cc_hook

    install_neuronx_cc_hook()
    nc = build_core_program()

    in_names = ["xt", "wqk", "wv", "wp"]
    out_avals = [jax.core.ShapedArray((N, DIM), np.float32)]
    partition_name = nc.partition_id_tensor.name if nc.partition_id_tensor else None

    def _body(xt, wqk, wvv, wp, yzero):
        operands = [xt, wqk, wvv, wp, yzero]
        names = in_names + ["y"]
        if nc.dbg_addr is not None:
            operands.append(np.zeros((1, 2), np.uint32))
            names.append(nc.dbg_addr.name)
        if partition_name is not None:
            operands.append(bass2jax.partition_id_tensor())
            names.append(partition_name)
        outs = _bass_exec_p.bind(
            *operands,
            out_avals=tuple(out_avals),
            in_names=tuple(names),
            out_names=("y",),
            lowering_input_output_aliases=(),
            sim_require_finite=True,
            sim_require_nnan=True,
            nc=nc,
        )
        return outs[0]

    devices = jax.devices()[:8]
    mesh = Mesh(np.asarray(devices), ("core",))
    fn = jax.jit(
        shard_map(
            _body, mesh=mesh,
            in_specs=(PartitionSpec("core"),) * 5,
            out_specs=PartitionSpec("core"),
            check_rep=False,
        ),
        keep_unused=True,
    )
    _CACHED["fn"] = fn
    return fn


def _run(in_maps):
    import jax

    fn = _get_fn()
    concat_in = [
        np.concatenate([m[name] for m in in_maps], axis=0)
        for name in ["xt", "wqk", "wv", "wp"]
    ]
    yzero = np.zeros((8 * N, DIM), np.float32)
    out = jax.block_until_ready(fn(*concat_in, yzero))
    return np.asarray(out).reshape(8, N, DIM)


def kernel(x, w_qkv, w_proj, b_proj):
    x = np.asarray(x, dtype=np.float32)
    w_qkv = np.asarray(w_qkv, dtype=np.float32)
    w_proj = np.asarray(w_proj, dtype=np.float32)
    b_proj = np.asarray(b_proj, dtype=np.float32)

    in_maps = _host_prep(x, w_qkv, w_proj)
    parts = _run(in_maps)

    y = np.empty((B, N, DIM), dtype=np.float32)
    for b in range(B):
        y[b] = parts[2 * b] + parts[2 * b + 1] + b_proj
    return y


# revision 8
# speedup vs baseline: 1.5709x; 1.4257x over previous
"""V2 Trainium2 Bass kernel for nn_Attention. See kernel.py for the math.

V2 structural changes vs V1:
- AV matmuls interleaved into the QK^T/exp chunk stream (PE no longer blocks
  on ACT; ACT has no tail idle).
- Score chunks CH=2 ([128,1024] psum, 2 banks) => banks: scores 2x2, AV 2,
  misc 2.
- qkv projection / output projection emitted as small "filler" groups spread
  across chunk steps.
"""

import sys

if "/opt/trn_rl_repo" not in sys.path:
    sys.path.insert(0, "/opt/trn_rl_repo")

import numpy as np
import ml_dtypes

import concourse.bacc as bacc
import concourse.mybir as mybir
import concourse.tile as tile
from concourse.masks import make_identity

FP32 = mybir.dt.float32
BF16 = mybir.dt.bfloat16
AF = mybir.ActivationFunctionType

DIM = 768
HEAD_DIM = 64
SCALE = HEAD_DIM ** -0.5
B, N = 4, 2048
HG = 6
CC = DIM // 128
PAIRS = HG // 2
S = N // 512
J = N // 128
CH = 2                      # kv blocks per score chunk
NCHUNK = J // CH            # 8 chunks per (head, strip)

_CACHED = {}


def build_core_program(reps=0, ablate=()):
    nc = bacc.Bacc("TRN2", debug=False, target_bir_lowering=False, num_devices=1)

    xt_d = nc.dram_tensor("xt", [DIM, N], BF16, kind="ExternalInput")
    wqk_d = nc.dram_tensor("wqk", [DIM, DIM], BF16, kind="ExternalInput")
    wv_d = nc.dram_tensor("wv", [DIM, HG * 64], BF16, kind="ExternalInput")
    wp_d = nc.dram_tensor("wp", [HG * 64, DIM], BF16, kind="ExternalInput")
    y_d = nc.dram_tensor("y", [N, DIM], FP32, kind="ExternalOutput")

    with tile.TileContext(nc) as tc:
        with (
            tc.tile_pool(name="persist", bufs=1) as persist,
            tc.tile_pool(name="exps", bufs=22) as exps_pool,
            tc.tile_pool(name="attnt", bufs=5) as attnt_pool,
            tc.tile_pool(name="small", bufs=4) as small_pool,
            tc.tile_pool(name="ysb", bufs=3) as y_pool,
            tc.tile_pool(name="ps_score", bufs=2, space="PSUM") as ps_score,
            tc.tile_pool(name="ps_av", bufs=2, space="PSUM") as ps_av,
            tc.tile_pool(name="ps_misc", bufs=2, space="PSUM") as ps_misc,
        ):
            xT = persist.tile([128, CC, N], BF16)
            wqk = persist.tile([128, CC, DIM], BF16)
            wv = persist.tile([128, CC, HG * 64], BF16)
            wp = persist.tile([128, PAIRS, DIM], BF16)
            qkT = persist.tile([128, CC, N], BF16)
            v = persist.tile([128, J, HG, 65], BF16)
            ident = persist.tile([128, 128], BF16)

            xt_r = xt_d.ap().rearrange("(o p) n -> p o n", p=128)
            wqk_r = wqk_d.ap().rearrange("(o p) n -> p o n", p=128)
            wv_r = wv_d.ap().rearrange("(o p) n -> p o n", p=128)
            for cc in range(CC):
                nc.sync.dma_start(out=wqk[:, cc], in_=wqk_r[:, cc])
                nc.sync.dma_start(out=xT[:, cc], in_=xt_r[:, cc])
                nc.sync.dma_start(out=wv[:, cc], in_=wv_r[:, cc])
            nc.sync.dma_start(out=wp, in_=wp_d.ap().rearrange("(o p) n -> p o n", p=128))
            make_identity(nc, ident)
            nc.vector.memset(v, 1.0)

            # ---------- filler groups (each: one psum-group of work) ----------

            def qk_group(ot, s):
                ps = ps_misc.tile([128, 512], FP32, tag="m")
                for cc in range(CC):
                    nc.tensor.matmul(
                        ps,
                        wqk[:, cc, ot * 128 : ot * 128 + 128],
                        xT[:, cc, s * 512 : s * 512 + 512],
                        start=(cc == 0), stop=(cc == CC - 1),
                    )
                nc.vector.tensor_copy(out=qkT[:, ot, s * 512 : s * 512 + 512], in_=ps)

            def v_group(p, nt):
                psv = ps_misc.tile([128, 512], FP32, tag="m")
                for cc in range(CC):
                    nc.tensor.matmul(
                        psv[:, 0:128],
                        xT[:, cc, nt * 128 : nt * 128 + 128],
                        wv[:, cc, p * 128 : p * 128 + 128],
                        start=(cc == 0), stop=(cc == CC - 1),
                    )
                for h2 in range(2):
                    nc.vector.tensor_copy(
                        out=v[:, nt, 2 * p + h2, 0:64],
                        in_=psv[:, h2 * 64 : h2 * 64 + 64],
                    )

            def proj_group(s, attnT, nt, og, ow, ysb):
                psy = ps_misc.tile([128, 512], FP32, tag="m")
                for cc in range(PAIRS):
                    nc.tensor.matmul(
                        psy[:, 0:ow],
                        attnT[:, cc, nt * 128 : nt * 128 + 128],
                        wp[:, cc, og : og + ow],
                        start=(cc == 0), stop=(cc == PAIRS - 1),
                    )
                nc.vector.tensor_copy(out=ysb[:, og : og + ow], in_=psy[:, 0:ow])
                if og == 512:
                    row = s * 512 + nt * 128
                    nc.sync.dma_start(out=y_d.ap()[row : row + 128, :], in_=ysb)

            def qkv_pair_fillers(p):
                out = []
                for ot in (p, PAIRS + p):
                    for s in range(S):
                        out.append(lambda ot=ot, s=s: qk_group(ot, s))
                for nt in range(J):
                    out.append(lambda nt=nt: v_group(p, nt))
                return out

            def proj_fillers(s, attnT):
                out = []
                for nt in range(4):
                    ysb = y_pool.tile([128, DIM], FP32, tag="y")
                    for og, ow in ((0, 512), (512, 256)):
                        out.append(
                            lambda nt=nt, og=og, ow=ow, ysb=ysb:
                                proj_group(s, attnT, nt, og, ow, ysb)
                        )
                return out

            def body():
                # pair-0 projections up front (lead-in; ACT idle here)
                for f in qkv_pair_fillers(0):
                    f()

                filler = []
                attnT_tiles = {}
                for hp in range(PAIRS):
                    if hp + 1 < PAIRS:
                        filler.extend(qkv_pair_fillers(hp + 1))
                    for s in range(S):
                        if hp == 0:
                            at = attnt_pool.tile(
                                [128, PAIRS, 512], BF16, tag="attnT",
                                name=f"attnT{s}",
                            )
                            attnT_tiles[s] = at
                        attnT = attnT_tiles[s]

                        pav = {}
                        for h2 in range(2):
                            pav[h2] = ps_av.tile(
                                [128, 512], FP32, tag="av", name=f"pav{h2}"
                            )

                        expS = {0: [None] * NCHUNK, 1: [None] * NCHUNK}

                        def emit_qkt_exp(c, s=s, hp=hp, expS=expS):
                            for h2, base in ((0, 0), (1, 64)):
                                pss = ps_score.tile(
                                    [128, 512 * CH], FP32, tag="sc"
                                )
                                for jj in range(CH):
                                    j = c * CH + jj
                                    if "qkt" in ablate:
                                        continue
                                    nc.tensor.matmul(
                                        pss[:, jj * 512 : jj * 512 + 512],
                                        qkT[base : base + 64, PAIRS + hp,
                                            j * 128 : j * 128 + 128],
                                        qkT[base : base + 64, hp,
                                            s * 512 : s * 512 + 512],
                                        start=True, stop=True,
                                        tile_position=(base, 0),
                                    )
                                et = exps_pool.tile(
                                    [128, 512 * CH], BF16, tag="e"
                                )
                                if "exp" not in ablate:
                                    nc.scalar.activation(
                                        out=et, in_=pss, func=AF.Exp,
                                    )
                                else:
                                    nc.vector.memset(et[:, 0:4], 1.0)
                                expS[h2][c] = et

                        def emit_av(c, hp=hp, pav=pav, expS=expS):
                            if "av" in ablate:
                                return
                            for h2 in range(2):
                                h = 2 * hp + h2
                                et = expS[h2][c]
                                for jj in range(CH):
                                    j = c * CH + jj
                                    nc.tensor.matmul(
                                        pav[h2][0:65, :],
                                        v[:, j, h, :],
                                        et[:, jj * 512 : jj * 512 + 512],
                                        start=(j == 0), stop=(j == J - 1),
                                    )

                        for c in range(NCHUNK):
                            emit_qkt_exp(c)
                            if c > 0:
                                emit_av(c - 1)
                            if filler:
                                filler.pop(0)()
                        emit_av(NCHUNK - 1)

                        # copy AV out of PSUM right away (frees the bank for
                        # the next slot), then divide from SBUF off-path
                        for h2 in range(2):
                            osb = small_pool.tile([65, 512], FP32, tag="o")
                            nc.vector.tensor_copy(out=osb, in_=pav[h2][0:65, :])
                            dsb = small_pool.tile([1, 512], FP32, tag="d")
                            if "av" in ablate:
                                nc.vector.memset(dsb, 1.0)
                            else:
                                nc.vector.reciprocal(out=dsb, in_=osb[64:65, :])
                            rb = small_pool.tile([64, 512], FP32, tag="rb")
                            nc.gpsimd.partition_broadcast(rb, dsb, channels=64)
                            nc.vector.tensor_tensor(
                                attnT[h2 * 64 : h2 * 64 + 64, hp, :],
                                osb[0:64, :],
                                rb,
                                mybir.AluOpType.mult,
                            )

                        if hp == PAIRS - 1:
                            filler.extend(
                                proj_fillers(s, attnT_tiles.pop(s))
                            )

                # drain remaining fillers (tail projections)
                for f in filler:
                    f()

            from contextlib import nullcontext
            with (tc.For_i(0, reps, 1) if reps else nullcontext()):
                body()

    nc.compile()
    return nc


def _host_prep(x, w_qkv, w_proj):
    bf16 = ml_dtypes.bfloat16
    in_maps = []
    for c in range(8):
        b, hg = c // 2, c % 2
        r0 = 384 * hg
        wq = w_qkv[r0 : r0 + 384] * SCALE
        wk = w_qkv[768 + r0 : 768 + r0 + 384]
        wvv = w_qkv[1536 + r0 : 1536 + r0 + 384]
        wqk = np.concatenate([wq, wk], axis=0)
        in_maps.append({
            "xt": np.ascontiguousarray(x[b].T).astype(bf16),
            "wqk": np.ascontiguousarray(wqk.T).astype(bf16),
            "wv": np.ascontiguousarray(wvv.T).astype(bf16),
            "wp": np.ascontiguousarray(w_proj[:, r0 : r0 + 384].T).astype(bf16),
        })
    return in_maps


def _get_fn():
    if "fn" in _CACHED:
        return _CACHED["fn"]

    import jax
    from jax.sharding import Mesh, PartitionSpec
    from jax.experimental.shard_map import shard_map
    from concourse import bass2jax
    from concourse.bass2jax import _bass_exec_p, install_neuronx_cc_hook

    install_neuronx_cc_hook()
    nc = build_core_program()

    in_names = ["xt", "wqk", "wv", "wp"]
    out_avals = [jax.core.ShapedArray((N, DIM), np.float32)]
    partition_name = nc.partition_id_tensor.name if nc.partition_id_tensor else None

    def _body(xt, wqk, wvv, wp, yzero):
        operands = [xt, wqk, wvv, wp, yzero]
        names = in_names + ["y"]
        if nc.dbg_addr is not None:
            operands.append(np.zeros((1, 2), np.uint32))
            names.append(nc.dbg_addr.name)
        if partition_name is not None:
            operands.append(bass2jax.partition_id_tensor())
            names.append(partition_name)
        outs = _bass_exec_p.bind(
            *operands,
            out_avals=tuple(out_avals),
            in_names=tuple(names),
            out_names=("y",),
            lowering_input_output_aliases=(),
            sim_require_finite=True,
            sim_require_nnan=True,
            nc=nc,
        )
        return outs[0]

    devices = jax.devices()[:8]
    mesh = Mesh(np.asarray(devices), ("core",))
    fn = jax.jit(
        shard_map(
            _body, mesh=mesh,
            in_specs=(PartitionSpec("core"),) * 5,
            out_specs=PartitionSpec("core"),
            check_rep=False,
        ),
        keep_unused=True,
    )
    _CACHED["fn"] = fn
    return fn


def _run(in_maps):
    import jax

    fn = _get_fn()
    concat_in = [
        np.concatenate([m[name] for m in in_maps], axis=0)
        for name in ["xt", "wqk", "wv", "wp"]
    ]
    yzero = np.zeros((8 * N, DIM), np.float32)
    out = jax.block_until_ready(fn(*concat_in, yzero))
    return np.asarray(out).reshape(8, N, DIM)


def kernel(x, w_qkv, w_proj, b_proj):
    x = np.asarray(x, dtype=np.float32)
    w_qkv = np.asarray(w_qkv, dtype=np.float32)
    w_proj = np.asarray(w_proj, dtype=np.float32)
    b_proj = np.asarray(b_proj, dtype=np.float32)

    in_maps = _host_prep(x, w_qkv, w_proj)
    parts = _run(in_maps)

    y = np.empty((B, N, DIM), dtype=np.float32)
    for b in range(B):
        y[b] = parts[2 * b] + parts[2 * b + 1] + b_proj
    return y
